# revision 36
# baseline (speedup 1.0000x reference)
"""Trainium2 Bass kernel for batched per-sample expert matmul (MoE routing).

Computes y[n, i] = relu(b[idxs[n], i] + sum_o w[idxs[n], i, o] * x[n, o])
for x (8192, 256), idxs (8192,), w (64, 256, 256), b (64, 256).

Strategy
--------
Host side (numpy, cheap):
  * Stable-sort all 8192 samples by expert id, shard the *sorted* batch
    contiguously across the 8 cores (1024 samples each). Each core's
    samples then span only a handful of contiguous experts, so the
    per-core weight traffic is ~3 MB instead of the full 16 MB table.
  * Cut each core's samples into segments of <= 128 samples, one expert
    per segment, padded so every core runs the same NSEG segments
    (SPMD: one program, per-core data).
  * Pre-gather, per segment: the expert's weight matrix laid out for the
    PE (contraction dim on partitions) and its bias row.  Pre-transpose
    the sorted x so the contraction dim is on partitions.

Device side (one static Tile program, identical on all 8 cores):
  for each segment s:
      psum[m, i]  = 1 * b[i]                       (K=1 bias matmul)
      psum[m, i] += sum_p xT0[p, m] * w0[p, i]     (K-chunk 0)
      psum[m, i] += sum_p xT1[p, m] * w1[p, i]     (K-chunk 1)
      y[m, i]     = relu(psum[m, i])               (ACT, PSUM -> SBUF)

  This walrus build allows only ONE semaphore wait on lowered matmul
  (LDWEIGHTS+MATMUL) and HWDGE direct-DMA instructions, so the program
  is shaped so nothing ever needs two: <= 8 DMAs total (no DMA-lane
  recycling), each weight batch in its own SBUF slot (no WAW waits),
  the bias matmul opens each PSUM group (operands resident; carries
  only the PSUM-release wait), K-chunk matmuls carry only the
  weight-DMA wait (x residency absorbed by a prologue dummy matmul),
  and ones/bias share one DMA.

Host side: scatter segment rows back to the original sample order.
Inputs with pathological expert skew can exceed the per-pass segment
budget; those run the same program over multiple passes.
"""

import os

import numpy as np

import concourse.bacc as bacc
import concourse.bass as bass
import concourse.mybir as mybir
import concourse.tile as tile
from concourse.bass_utils import run_bass_kernel_spmd

N_CORES = 8
P = 128          # SBUF/PSUM partitions
F = 256          # feature dim (in_features == out_features == 256)
SEG = 128        # samples per segment (== max PSUM partition dim)
OGS = 4          # segments per output-DMA batch
MAX_NSEG = 32    # per-pass segment budget (SBUF residency bound)


def _batches(n, sizes, rest):
    """Split range(n) into batches: explicit `sizes` first, then `rest`-sized."""
    out = []
    lo = 0
    i = 0
    while lo < n:
        sz = sizes[i] if i < len(sizes) else rest
        i += 1
        hi = min(n, lo + sz)
        out.append((lo, hi))
        lo = hi
    return out
MM_DT = (
    mybir.dt.float32
    if os.environ.get("KBENCH_MM_DT", "float32r") == "float32"
    else mybir.dt.float32r
)  # matmul operand dtype; float32r streams 4x faster at moving dim >= 256

# Set by the last kernel() call when KBENCH_TRACE=1 (used by test.py only).
LAST_EXEC_TIME_NS = None
LAST_TRACE = None


def _build_schedule(idxs: np.ndarray):
    """Sort samples by expert, shard, and cut per-core single-expert segments."""
    B = idxs.shape[0]
    S = B // N_CORES
    order = np.argsort(idxs, kind="stable")
    sidx = idxs[order]
    per_core = []
    for c in range(N_CORES):
        e = sidx[c * S:(c + 1) * S]
        # run-length encode the (sorted) expert ids of this core's chunk
        segs = []  # (expert, local_start, count), count <= SEG
        i = 0
        while i < S:
            j = i
            while j < S and e[j] == e[i]:
                j += 1
            k = i
            while k < j:
                cnt = min(SEG, j - k)
                segs.append((int(e[i]), k, cnt))
                k += cnt
            i = j
        per_core.append(segs)
    return order, per_core


def _build_program(nseg: int):
    nc = bacc.Bacc(
        "TRN2", target_bir_lowering=False, debug=False, num_devices=N_CORES
    )
    npad = nseg * SEG
    xt_d = nc.dram_tensor("xt", [P, 2, npad], MM_DT, kind="ExternalInput").ap()
    w_d = nc.dram_tensor("wseg", [P, nseg, 2 * F], MM_DT, kind="ExternalInput").ap()
    b_d = nc.dram_tensor(
        "bconst", [1, P + nseg * F], MM_DT, kind="ExternalInput"
    ).ap()
    y_d = nc.dram_tensor(
        "y", [P, nseg, F], mybir.dt.float32, kind="ExternalOutput"
    ).ap()

    f32 = mybir.dt.float32
    relu = mybir.ActivationFunctionType.Relu

    # Geometric batches: tiny first transfers so compute starts early, then
    # large ones to amortize the per-DMA fixed cost on the SWDGE queue.
    # x-batch starts (2, 6, 14, 22, ...) are staggered against w-batch
    # starts (1, 3, 7, 13, 19, ...) so no K-chunk matmul ever needs both an
    # x-DMA and a w-DMA wait (lowered matmuls carry a single wait; segment
    # 0 is covered by the prologue dummy matmul).
    wbat = _batches(nseg, [1, 2, 4], 6)
    xbat = _batches(nseg, [2, 4], 8)

    with tile.TileContext(nc) as tc:
        with (
            tc.tile_pool(name="const", bufs=1) as const,
            tc.tile_pool(name="w", bufs=1) as wpool,
            tc.tile_pool(name="yout", bufs=2) as ypool,
            tc.tile_pool(name="ps", bufs=3, space="PSUM") as pspool,
            tc.tile_pool(name="scr", bufs=1, space="PSUM") as scrpool,
        ):
            # bconst rides the otherwise-idle HWDGE ring; the x/w streams go
            # through the single SWDGE queue (gpsimd): FIFO delivery in issue
            # order at full line rate, one completion semaphore per batch ->
            # a just-in-time pipeline.  (Independent HWDGE queues share SDMA
            # bandwidth round-robin, which delays the earliest transfer.)
            bc = const.tile([1, P + nseg * F], MM_DT, tag="bconst")
            nc.sync.dma_start(bc[:], b_d[:])

            xts = {}

            def load_x_batch(b):
                lo, hi = xbat[b]
                t = const.tile([P, 2 * (hi - lo) * SEG], MM_DT, tag=f"xt{b}")
                xts[b] = t
                nc.gpsimd.dma_start(
                    t[:].rearrange("p (c n) -> p c n", c=2),
                    xt_d[:, :, lo * SEG:hi * SEG],
                )

            wts = {}

            def load_w_batch(g):
                lo, hi = wbat[g]
                t = wpool.tile([P, (hi - lo) * 2 * F], MM_DT, tag=f"w{g}")
                wts[g] = t
                nc.gpsimd.dma_start(
                    t[:], w_d[:, lo:hi, :].rearrange("p g f -> p (g f)")
                )

            ones = bc[:, 0:P]

            seg2x = {}
            for b, (lo, hi) in enumerate(xbat):
                for s in range(lo, hi):
                    seg2x[s] = b
            seg2w = {}
            for g, (lo, hi) in enumerate(wbat):
                for s in range(lo, hi):
                    seg2w[s] = g

            def xchunk(s, c):
                b = seg2x[s]
                lo, hi = xbat[b]
                base = (c * (hi - lo) + (s - lo)) * SEG
                return xts[b][:, base:base + SEG]

            def wchunk(s, c):
                g = seg2w[s]
                lo, hi = wbat[g]
                base = (2 * (s - lo) + c) * F
                return wts[g][:, base:base + F]

            # need-order emission on the SWDGE queue
            nxt_x = 0
            for g in range(len(wbat)):
                while nxt_x < len(xbat) and xbat[nxt_x][0] <= wbat[g][0]:
                    load_x_batch(nxt_x)
                    nxt_x += 1
                load_w_batch(g)
            while nxt_x < len(xbat):
                load_x_batch(nxt_x)
                nxt_x += 1

            scr = scrpool.tile([2, 2], f32)
            yt = None
            # Process segments in pairs sharing one full PSUM bank: a single
            # N=512 bias matmul covers both, and one ACT relu drains both.
            pairs = _batches(nseg, [], 2)
            for plo, phi in pairs:
                pw = (phi - plo) * F
                if plo % OGS == 0:
                    yt = ypool.tile([P, OGS * F], f32)
                if plo == 0:
                    # Absorb x batch 0's DMA wait into the PE's clock so
                    # segment 0's K-chunk matmul only needs the w-DMA wait.
                    xb = xts[0]
                    nc.tensor.matmul(
                        scr[:], xb[:, 0:2], xb[:, 0:2], start=True, stop=True
                    )
                ps = pspool.tile([P, 2 * F], f32)
                nc.tensor.matmul(
                    ps[:, 0:pw],
                    ones,
                    bc[:, P + plo * F:P + phi * F],
                    start=True,
                    stop=False,
                )
                for s in range(plo, phi):
                    o = (s - plo) * F
                    nc.tensor.matmul(
                        ps[:, o:o + F], xchunk(s, 0), wchunk(s, 0),
                        start=False, stop=False,
                    )
                    nc.tensor.matmul(
                        ps[:, o:o + F], xchunk(s, 1), wchunk(s, 1),
                        start=False, stop=(s == phi - 1),
                    )
                j = plo % OGS
                # relu on DVE: keeps ACT (and its 1.3us table-load preamble)
                # out of the kernel entirely.
                nc.vector.tensor_scalar_max(
                    yt[:, j * F:j * F + pw], ps[:, 0:pw], 0.0
                )
                if phi % OGS == 0 or phi == nseg:
                    lo = (plo // OGS) * OGS
                    nc.sync.dma_start(
                        y_d[:, lo:phi, :].rearrange("p g f -> p (g f)"),
                        yt[:, 0:(phi - lo) * F],
                    )
    nc.compile()
    return nc


def kernel(x: np.ndarray, idxs: np.ndarray, w: np.ndarray, b: np.ndarray) -> np.ndarray:
    global LAST_EXEC_TIME_NS, LAST_TRACE
    x = np.ascontiguousarray(x, dtype=np.float32)
    w = np.ascontiguousarray(w, dtype=np.float32)
    b = np.ascontiguousarray(b, dtype=np.float32)
    idxs_np = np.asarray(idxs).astype(np.int64)

    B = x.shape[0]
    S = B // N_CORES
    order, per_core = _build_schedule(idxs_np)

    # Split each core's segment list into passes of <= MAX_NSEG segments.
    npass = max(1, (max(len(s) for s in per_core) + MAX_NSEG - 1) // MAX_NSEG)
    if npass == 1:
        nseg = max(2, max(len(s) for s in per_core))
    else:
        nseg = MAX_NSEG
    npad = nseg * SEG

    # Per-expert weight blocks in PE layout:
    # wprep[e, p, c*F + i] = w[e, i, c*P + p]  (c = contraction chunk 0/1)
    wprep = np.ascontiguousarray(
        w.transpose(0, 2, 1)           # (e, o, i)
        .reshape(64, 2, P, F)          # (e, c, p, i)
        .transpose(0, 2, 1, 3)         # (e, p, c, i)
        .reshape(64, P, 2 * F)
    )

    nc = _build_program(nseg)
    trace = bool(os.environ.get("KBENCH_TRACE"))

    y = np.empty((B, F), dtype=np.float32)
    for pi in range(npass):
        in_maps = []
        for c in range(N_CORES):
            sel = order[c * S:(c + 1) * S]
            segs = per_core[c][pi * MAX_NSEG:(pi + 1) * MAX_NSEG]
            xpad = np.zeros((npad, F), dtype=np.float32)
            eids = np.zeros(nseg, dtype=np.int64)
            for s, (e, k0, cnt) in enumerate(segs):
                xpad[s * SEG:s * SEG + cnt] = x[sel[k0:k0 + cnt]]
                eids[s] = e
            # xt[p, c, n] = xpad[n, c*P + p]
            xt = np.ascontiguousarray(
                xpad.T.reshape(2, P, npad).transpose(1, 0, 2)
            )
            wseg = np.ascontiguousarray(
                wprep[eids].transpose(1, 0, 2)
            )  # (P, nseg, 2F)
            bconst = np.concatenate(
                [np.ones(P, dtype=np.float32), b[eids].reshape(nseg * F)]
            ).reshape(1, P + nseg * F)
            in_maps.append({"xt": xt, "wseg": wseg, "bconst": bconst})

        res = run_bass_kernel_spmd(
            nc, in_maps, core_ids=list(range(N_CORES)), trace=trace
        )
        LAST_EXEC_TIME_NS = res.exec_time_ns
        LAST_TRACE = res.instructions_and_trace

        for c in range(N_CORES):
            sel = order[c * S:(c + 1) * S]
            segs = per_core[c][pi * MAX_NSEG:(pi + 1) * MAX_NSEG]
            ypad = res.results[c]["y"].transpose(1, 0, 2).reshape(npad, F)
            for s, (e, k0, cnt) in enumerate(segs):
                y[sel[k0:k0 + cnt]] = ypad[s * SEG:s * SEG + cnt]
    return y
